# revision 1
# baseline (speedup 1.0000x reference)
"""AutoCorrelation (FFT-free) kernel for 8 Trainium2 NeuronCores.

Math: the reference computes, per (b, h, e), the circular cross-correlation
corr = irfft(rfft(q) * conj(rfft(k))), then
  mean_corr[b, l] = mean_{h,e} corr          (only this is ever used)
  global_mean[l]  = mean_b mean_corr
  topk lags       = top-7 of global_mean
  weights         = softmax(mean_corr[:, topk])
  out[b,l]        = sum_k w[b,k] * v[b, (l - lag_k) % L]

Identity used: mean_corr[b, l] = (1/HE) * sum_s <q[b,(s+l)%L,:,:], k[b,s,:,:]>.
So instead of FFTs we compute, per batch, the Gram matrix G[s,t] = sum_c
kT[c,s] qT[c,t] on the TensorEngine (fp16 inputs, fp32 PSUM accumulation) and
fold its wrapped diagonals. The fold is fused into the matmul by *rotating
each s-chunk's output columns in PSUM* (write column t of chunk u at
y=(t-128u)%L, accumulating): afterwards mean_corr[l] = sum_p S[p,(l+p)%L],
which a 7-level rotate-add tree reduces in O(L*128) vector work.

Sharding: batch across the 8 cores (2 per core). Only global_mean needs an
AllReduce of a [1,1536] fp32 vector. Top-7 via the DVE max/max_index
instruction. The topk lags become PE registers driving dynamic rhs
access-pattern offsets: the weighted circular gather-sum runs as w-scaled
identity matmuls reading shifted windows of a doubled v buffer, accumulated
in PSUM (start/stop per 7-tap group). This needs the bacc layer: each
dynamic-offset instruction consumes registers at lowering, and only bacc's
graph-coloring allocator keeps 168 dynamic matmuls within the 49-register
file. A staged-copy fallback (DYN_OUTPUT=False) splits the dynamic work
into a few large ACT/DVE copies instead.

fp16 is safe here: top-7 global_mean gap is 1.5e-3 while fp16-input error is
<5e-4 (validated against the fp32 FFT reference), and the output tolerance is
2e-2 vs our 7e-4.
"""

import numpy as np

B, L, H, E = 16, 1536, 8, 64
C = H * E             # 512 channels = H*E
NCORES = 8
BLOC = B // NCORES    # batches per core
NCC = C // 128        # channel chunks of 128
TOPK = 7              # int(1 * log(1536)) == 7
NJ = L // 128         # s-chunks
NLT = L // 512        # output l-tiles

_cache = {}
DEBUG_BUILD = False
STG_BUFS = 7          # number of rotating stage-tile tags
SKIP_OUTPUT = False   # timeline experiments
SKIP_GRAM = False
SKIP_TREE = False
DYN_OUTPUT = True


def _build(num_cores: int):
    import concourse.bass as bass
    import concourse.bacc as bacc
    import concourse.mybir as mybir
    import concourse.tile as tile

    f16 = mybir.dt.float16
    f32 = mybir.dt.float32
    u32 = mybir.dt.uint32
    PE = mybir.EngineType.PE
    ACT = mybir.EngineType.Activation

    nc = bacc.Bacc(None)
    qT = nc.dram_tensor("qT", [BLOC, C, L], f16, kind="ExternalInput")
    kT = nc.dram_tensor("kT", [BLOC, C, L], f16, kind="ExternalInput")
    vT = nc.dram_tensor("vT", [BLOC, C, L], f16, kind="ExternalInput")
    out = nc.dram_tensor("out", [BLOC, C, L], f16, kind="ExternalOutput")
    if DEBUG_BUILD:
        dbg_i = nc.dram_tensor("dbg_i", [1, 8], u32, kind="ExternalOutput")
        dbg_f = nc.dram_tensor("dbg_f", [BLOC, 48], f32, kind="ExternalOutput")
        dbg_gm = nc.dram_tensor("dbg_gm", [1, L], f32, kind="ExternalOutput")
    ident_d = nc.inline_tensor(np.eye(128, dtype=np.float16), "identc")
    ones_d = nc.inline_tensor(np.ones((1, 128), np.float16), "onesc")

    with tile.TileContext(nc) as tc:
        with (
            tc.tile_pool(name="sb", bufs=1) as sb,
            tc.tile_pool(name="sps", bufs=1, space="PSUM") as sps,
            tc.tile_pool(name="ops", bufs=1, space="PSUM") as ops,
            tc.tile_pool(name="obp", bufs=3) as obp,
            tc.tile_pool(name="dram", bufs=1, space="DRAM") as dram,
        ):
            ident = sb.tile([128, 128], f16, tag="ident")
            ones = sb.tile([1, 128], f16, tag="ones")
            nc.sync.dma_start(ident, ident_d[:])
            nc.sync.dma_start(ones, ones_d[:])

            # ---- doubled v buffers (independent loads; overlap with Gram) ----
            # layout per batch: [128, NCC, 2L] so one dynamic-offset copy can
            # gather a circular window for several channel chunks at once
            vv = []
            for bi in range(BLOC):
                t = sb.tile([128, NCC, 2 * L], f16, tag=f"vv{bi}")
                for cc in range(NCC):
                    nc.scalar.dma_start(t[:, cc, 0:L], vT[bi, 128 * cc:128 * (cc + 1), :])
                # duplicate the halves on the VectorEngine instead of re-reading HBM
                nc.vector.tensor_copy(t[:, :, L:2 * L], t[:, :, 0:L])
                vv.append(t)

            # ---- Gram with rotated PSUM accumulation + diagonal fold ----
            mc = []  # per-batch mean_corr [1, L] fp32
            for bi in range(BLOC):
                ks, qs = [], []
                for cc in range(NCC):
                    t = sb.tile([128, L], f16, tag=f"k{bi}{cc}")
                    nc.sync.dma_start(t, kT[bi, 128 * cc:128 * (cc + 1), :])
                    ks.append(t)
                for cc in range(NCC):
                    t = sb.tile([128, L], f16, tag=f"q{bi}{cc}")
                    nc.sync.dma_start(t, qT[bi, 128 * cc:128 * (cc + 1), :])
                    qs.append(t)

                S = sps.tile([128, L], f32, tag=f"S{bi}")
                for u in range(NJ):
                    r = (L - 128 * u) % L
                    segs = []
                    t0 = 0
                    while t0 < L:
                        y0 = (t0 + r) % L
                        seg = min(512 - (y0 % 512), L - t0, L - y0)
                        segs.append((t0, y0, seg))
                        t0 += seg
                    for cc in range(NCC):
                        if SKIP_GRAM and not (u == 0 and cc == 0) and not (u == NJ - 1 and cc == NCC - 1):
                            continue
                        for (ts_, ys_, seg) in segs:
                            nc.tensor.matmul(
                                S[:, ys_:ys_ + seg],
                                ks[cc][:, 128 * u:128 * (u + 1)],
                                qs[cc][:, ts_:ts_ + seg],
                                start=(u == 0 and cc == 0),
                                stop=(u == NJ - 1 and cc == NCC - 1),
                                skip_group_check=True,
                            )

                # Evict PSUM in four 32-partition chunks, all to partition
                # base 0 (legal for ACT, unlike engine tensor ops), with the
                # 1/HE normalization fused in. This turns the first two fold
                # levels into pure free-dim rotated adds with no DMA.
                T01 = sb.tile([32, 2, L], f32, tag="Ssb01")
                T23 = sb.tile([32, 2, L], f32, tag="Ssb23")
                for aa in range(4):
                    tdst = T01 if aa < 2 else T23
                    nc.scalar.mul(tdst[:, aa % 2, :], S[32 * aa:32 * (aa + 1), :], 1.0 / C)

                if SKIP_TREE:
                    mctile = sb.tile([1, L], f32, tag=f"mc{bi}")
                    nc.scalar.copy(mctile, T01[0:1, 0, :])
                    mc.append(mctile)
                    continue
                # fold: mean_corr[l] = sum_p S[p, (l+p)%L]; with p = 32a + b:
                # U[b, y] = sum_a T[b, a, (y + 32a)%L], then the rotate-add
                # tree continues over b. Two eviction tiles so each combine
                # add only waits on its own pair of evictions.
                A = sb.tile([32, L], f32, tag="trA")
                nc.vector.tensor_add(A[:, 0:L - 32], T01[:, 0, 0:L - 32], T01[:, 1, 32:L])
                nc.vector.tensor_add(A[:, L - 32:L], T01[:, 0, L - 32:L], T01[:, 1, 0:32])
                Bt = sb.tile([32, L], f32, tag="trB")
                nc.vector.tensor_add(Bt[:, 0:L - 32], T23[:, 0, 0:L - 32], T23[:, 1, 32:L])
                nc.vector.tensor_add(Bt[:, L - 32:L], T23[:, 0, L - 32:L], T23[:, 1, 0:32])
                U = sb.tile([32, L], f32, tag="trU")
                nc.vector.tensor_add(U[:, 0:L - 64], A[:, 0:L - 64], Bt[:, 64:L])
                nc.vector.tensor_add(U[:, L - 64:L], A[:, L - 64:L], Bt[:, 0:64])

                cur = U
                h = 16
                pp = 0
                while h >= 1:
                    if h == 1:
                        nxt = sb.tile([1, L], f32, tag=f"mc{bi}")
                    else:
                        nxt = sb.tile([h, L], f32, tag=("trA" if pp == 0 else "trB"))
                    # engines require equal start partitions on all operands:
                    # realign the upper half via one plain DMA, then apply the
                    # rotation through free-dim offsets in the adds
                    tmp = sb.tile([h, L], f32, tag="tt0")
                    nc.sync.dma_start(tmp, cur[h:2 * h, :])
                    nc.vector.tensor_add(nxt[:, 0:L - h], cur[0:h, 0:L - h], tmp[:, h:L])
                    nc.vector.tensor_add(nxt[:, L - h:L], cur[0:h, L - h:L], tmp[:, 0:h])
                    cur = nxt
                    pp ^= 1
                    h //= 2
                mc.append(cur)

            # ---- global mean: local batch sum + cross-core AllReduce ----
            # (also stage mean_corr rows in DRAM for the dynamic weight gathers)
            mc_dram = dram.tile([BLOC, L], f32)
            for bi in range(BLOC):
                nc.sync.dma_start(mc_dram[bi:bi + 1, :], mc[bi])
            gl = sb.tile([1, L], f32, tag="gl")
            nc.vector.tensor_add(gl, mc[0], mc[1])
            if num_cores > 1:
                cc_in = dram.tile([1, L], f32)
                cc_out = dram.tile([1, L], f32)
                nc.sync.dma_start(cc_in, gl)
                nc.gpsimd.collective_compute(
                    "AllReduce",
                    mybir.AluOpType.add,
                    replica_groups=[list(range(num_cores))],
                    ins=[cc_in.opt()],
                    outs=[cc_out.opt()],
                )
                gm = sb.tile([1, L], f32, tag="gm")
                nc.sync.dma_start(gm, cc_out)
            else:
                gm = gl

            # ---- top-7 lags (top-8 instruction, first 7 used) ----
            vals = sb.tile([1, 8], f32, tag="vals")
            idxs = sb.tile([1, 8], u32, tag="idxs")
            nc.vector.max(vals, gm)
            nc.vector.max_index(idxs, vals, gm)

            # index registers. Each dynamic-offset instruction consumes an
            # engine register at lowering, so the shifted reads are done as a
            # few big "weighted staging" copies on ACT/DVE (k split by parity)
            # and all matmuls stay static.
            act_eng = nc.engines[ACT]
            dve_eng = nc.engines[mybir.EngineType.DVE]
            gp_eng = nc.engines[mybir.EngineType.Pool]
            pe_eng = nc.engines[PE]
            DVE_KS = (0, 2, 4, 6)   # ACT: (1, 3, 5)  (staged fallback path)
            GP_KS = ()
            sv_gi = []   # GpSimd: raw idx for weight gathers (via dynamic DMA)
            sv_o = {}    # per tap: shift o_k = L - idx_k on the staging engine
            sv_x = {}    # PE: o_k + 512*lt for the dynamic-matmul path
            for k in range(TOPK):
                ra = gp_eng.alloc_register(f"ia{k}")
                gp_eng.reg_load(ra, idxs[0:1, k:k + 1])
                sv_g = gp_eng.snap(ra, donate=True, min_val=0, max_val=L - 1)
                sv_gi.append(sv_g)
                if DYN_OUTPUT:
                    rp = pe_eng.alloc_register(f"ip{k}")
                    pe_eng.reg_load(rp, idxs[0:1, k:k + 1])
                    ro = pe_eng.alloc_register(f"io{k}")
                    pe_eng.reg_alu(ro, L, rp, mybir.AluOpType.subtract)
                    for lt in range(NLT):
                        rx = pe_eng.alloc_register(f"ix{k}_{lt}")
                        pe_eng.reg_alu(rx, ro, 512 * lt, mybir.AluOpType.add)
                        sv_t = pe_eng.snap(rx, donate=True, min_val=1, max_val=L + 1024)
                        sv_x[(k, lt)] = sv_t
                    continue
                eng = dve_eng if k in DVE_KS else act_eng
                rp = eng.alloc_register(f"ip{k}")
                eng.reg_load(rp, idxs[0:1, k:k + 1])
                ro = eng.alloc_register(f"io{k}")
                eng.reg_alu(ro, L, rp, mybir.AluOpType.subtract)
                sv_t = eng.snap(ro, donate=True, min_val=1, max_val=L)
                sv_o[k] = sv_t

            # ---- softmax weights for both batches, broadcast to partitions ----
            wr = sb.tile([BLOC, 8], f32, tag="wr")
            for k in range(TOPK):
                nc.gpsimd.dma_start(wr[:, k:k + 1], mc_dram[:, bass.ds(sv_gi[k], 1)])
            # no max-subtraction: gathered mean_corr values are bounded
            # (|x| < ~8), fp32 exp is safe, and softmax is shift-invariant --
            # saves three serial ops on the post-collective critical path
            ex = sb.tile([BLOC, 8], f32, tag="ex")
            nc.scalar.activation(
                ex[:, 0:TOPK], wr[:, 0:TOPK],
                mybir.ActivationFunctionType.Exp,
                bias=0.0, scale=1.0,
            )
            sm = sb.tile([BLOC, 1], f32, tag="sm")
            nc.vector.reduce_sum(sm, ex[:, 0:TOPK], axis=mybir.AxisListType.X)
            rs = sb.tile([BLOC, 1], f32, tag="rs")
            nc.vector.reciprocal(rs, sm)
            w16 = sb.tile([BLOC, 8], f16, tag="w16")
            nc.scalar.mul(w16[:, 0:TOPK], ex[:, 0:TOPK], rs[:, 0:1])
            # row 1 must move to a base-0 tile to be a legal matmul operand
            w16b = sb.tile([1, 8], f16, tag="w16b")
            nc.sync.dma_start(w16b[:, 0:TOPK], w16[1:2, 0:TOPK])
            wbcs = []
            for bi in range(BLOC):
                wps = ops.tile([128, 8], f32, tag="wps")
                wsrc = w16[0:1, 0:TOPK] if bi == 0 else w16b[0:1, 0:TOPK]
                nc.tensor.matmul(wps[:, 0:TOPK], ones, wsrc, start=True, stop=True)
                wbc = sb.tile([128, 8], f32, tag=f"wbc{bi}")
                nc.scalar.copy(wbc[:, 0:TOPK], wps[:, 0:TOPK])
                wbcs.append(wbc)

            if DEBUG_BUILD:
                nc.sync.dma_start(dbg_i[:], idxs)
                nc.sync.dma_start(dbg_gm[:], gm)
                for bi in range(BLOC):
                    nc.sync.dma_start(dbg_f[bi:bi + 1, 0:8], wr[bi:bi + 1, :])
                    nc.sync.dma_start(dbg_f[bi:bi + 1, 8:16], ex[bi:bi + 1, :])
                    nc.sync.dma_start(dbg_f[bi:bi + 1, 16:24], wbcs[bi][0:1, :])
                    nc.sync.dma_start(dbg_f[bi:bi + 1, 24:32], wbcs[bi][64:65, :])
            # ---- weighted circular gather-sum ----
            # stage w_k * v[:, cc_pair, o_k : o_k+L] (dynamic offset, ACT/DVE),
            # then accumulate the 7 taps with static identity matmuls in PSUM.
            # PSUM reuses the Gram S tags (3 banks each = 3 l-slices).
            if DYN_OUTPUT and not SKIP_OUTPUT:
                # w-scaled identity weights, one per (batch, tap)
                Iw = [[None] * TOPK for _ in range(BLOC)]
                for bi in range(BLOC):
                    for k in range(TOPK):
                        t = sb.tile([128, 128], f16, tag=f"iw{bi}{k}")
                        nc.vector.tensor_scalar_mul(t, ident, wbcs[bi][:, k:k + 1])
                        Iw[bi][k] = t
                for bi in range(BLOC):
                    for cc in range(NCC):
                        # one 3-bank PSUM tag per channel chunk, alternating
                        # S0/S1 so chunk n+1's matmuls overlap chunk n's
                        # evictions. Slice-outer, taps inner: each [128,512]
                        # slice finishes its 7-tap accumulation early so its
                        # eviction and out-DMA stream behind the TensorEngine.
                        tgt = sps.tile([128, L], f32, tag=("S0" if cc % 2 == 0 else "S1"))
                        ot = obp.tile([128, L], f16, tag="ot")
                        for lt in range(NLT):
                            for k in range(TOPK):
                                nc.tensor.matmul(
                                    tgt[:, 512 * lt:512 * (lt + 1)],
                                    Iw[bi][k],
                                    vv[bi][:, cc, bass.ds(sv_x[(k, lt)], 512)],
                                    start=(k == 0),
                                    stop=(k == TOPK - 1),
                                    skip_group_check=True,
                                )
                            nc.scalar.copy(
                                ot[:, 512 * lt:512 * (lt + 1)],
                                tgt[:, 512 * lt:512 * (lt + 1)],
                            )
                            nc.sync.dma_start(
                                out[bi, 128 * cc:128 * (cc + 1), 512 * lt:512 * (lt + 1)],
                                ot[:, 512 * lt:512 * (lt + 1)],
                            )
            for bi in range(BLOC):
                if SKIP_OUTPUT or DYN_OUTPUT:
                    break
                vvr = vv[bi]
                for ccg in range(NCC // 2):
                    poA = sps.tile([128, L], f32, tag="S0")
                    poB = sps.tile([128, L], f32, tag="S1")
                    for k in range(TOPK):
                        stg_tags = ["q00", "q01", "q02", "q03", "q10", "q11", "q12"]
                        stg = sb.tile([128, 2, L], f16, tag=stg_tags[k % STG_BUFS])
                        src = vvr[:, 2 * ccg:2 * ccg + 2, bass.ds(sv_o[k], L)]
                        if k in DVE_KS:
                            nc.vector.tensor_scalar_mul(stg, src, wbcs[bi][:, k:k + 1])
                        elif k in GP_KS:
                            nc.gpsimd.tensor_scalar_mul(stg, src, wbcs[bi][:, k:k + 1])
                        else:
                            nc.scalar.mul(stg, src, wbcs[bi][:, k:k + 1])
                        for cc2 in range(2):
                            tgt = poA if cc2 == 0 else poB
                            for lt in range(NLT):
                                nc.tensor.matmul(
                                    tgt[:, 512 * lt:512 * (lt + 1)],
                                    ident,
                                    stg[:, cc2, 512 * lt:512 * (lt + 1)],
                                    start=(k == 0),
                                    stop=(k == TOPK - 1),
                                    skip_group_check=True,
                                )
                    for cc2 in range(2):
                        cc = 2 * ccg + cc2
                        ot = obp.tile([128, L], f16, tag="ot")
                        nc.scalar.copy(ot, poA if cc2 == 0 else poB)
                        nc.sync.dma_start(out[bi, 128 * cc:128 * (cc + 1), :], ot)
    nc.finalize()
    return nc


def _marshal(arr, ncores):
    # [B, L, H, E] fp32 -> per-core contiguous fp16 [BLOC, C, L]
    a = arr.reshape(B, L, C).astype(np.float16)
    a = np.ascontiguousarray(a.transpose(0, 2, 1))  # [B, C, L]
    bloc = B // ncores
    return [a[c * bloc:(c + 1) * bloc] for c in range(ncores)]


def _ensure_axon_hooks_importable():
    # some containers lack antenv.axon_hooks; run_bass_kernel_spmd imports it
    # unconditionally when tracing is requested. A None hook degrades to an
    # untraced run instead of crashing.
    import sys
    import types
    try:
        import antenv.axon_hooks  # noqa: F401
    except ModuleNotFoundError:
        try:
            import antenv
        except ModuleNotFoundError:
            return
        m = types.ModuleType("antenv.axon_hooks")
        m.get_axon_ntff_profile_hook = lambda: None
        sys.modules["antenv.axon_hooks"] = m
        antenv.axon_hooks = m


def kernel(queries, keys, values, attn_mask=None, _trace=False):
    from concourse.bass_utils import run_bass_kernel_spmd

    _ensure_axon_hooks_importable()

    nc = _cache.get("nc")
    if nc is None:
        nc = _build(NCORES)
        _cache["nc"] = nc

    qs = _marshal(np.asarray(queries, np.float32), NCORES)
    ks = _marshal(np.asarray(keys, np.float32), NCORES)
    vs = _marshal(np.asarray(values, np.float32), NCORES)
    in_maps = [{"qT": qs[c], "kT": ks[c], "vT": vs[c]} for c in range(NCORES)]

    res = run_bass_kernel_spmd(nc, in_maps, core_ids=list(range(NCORES)), trace=_trace)
    _cache["last"] = res
    o = np.concatenate([res.results[c]["out"] for c in range(NCORES)], axis=0)
    o = o.transpose(0, 2, 1).astype(np.float32)  # [B, L, C]
    return np.ascontiguousarray(o.reshape(B, L, H, E))



# revision 26
# speedup vs baseline: 1.4160x; 1.4160x over previous
"""AutoCorrelation (FFT-free) kernel for 8 Trainium2 NeuronCores.

Math: the reference computes, per (b, h, e), the circular cross-correlation
corr = irfft(rfft(q) * conj(rfft(k))), then
  mean_corr[b, l] = mean_{h,e} corr          (only this is ever used)
  global_mean[l]  = mean_b mean_corr
  topk lags       = top-7 of global_mean
  weights         = softmax(mean_corr[:, topk])
  out[b,l]        = sum_k w[b,k] * v[b, (l - lag_k) % L]

Identity used: mean_corr[b, l] = (1/HE) * sum_s <q[b,(s+l)%L,:,:], k[b,s,:,:]>.
Per batch the Gram matrix G[s,t] = sum_c kT[c,s] qT[c,t] runs on the
TensorEngine (fp16 inputs, fp32 PSUM accumulation), with each s-chunk's
output columns rotated in PSUM so that afterwards
mean_corr[l] = sum_p S[p, (l+p)%L]. The channel loop is outermost so the
first (k, q) chunk pair covers 12 s-blocks (~7us) of matmul per load.

The diagonal fold runs as: evict S to a doubled fp16 SBUF tile (1/HE scale
fused), a diagonal-access-pattern DMA (per-partition +1 element skew,
expressible because DMA APs are flat element strides), then a ones-vector
matmul reduces the 128 partitions in PSUM (512-col chunks reusing the Gram
banks). Eviction + diagonal DMA are split in two column chunks to pipeline.
global_mean accumulates both batches' diagonals into one PSUM group on the
otherwise-idle PE. This replaces the previous 7-level rotate-add fold tree
(~25us of serial DVE/DMA ops) with ~4us that mostly hides under the Gram.

Top-7 via the DVE max/max_index top-8 instruction. Weights: dynamic-offset
ACT/DVE element copies gather mean_corr[b, lag_k] straight from SBUF (no
SWDGE round trips), softmax per batch at partition 0 (no cross-partition
DMA hop).

Output gather-sum sum_k w_k v[(l-lag_k)%L], cost-model-balanced across
engines (PE identity-matmul tap = 2.56us, ACT stage = 5.3us, DVE fused
tap = 6.5us, DVE fused eviction slice = 0.66us):
  batch 0: taps 0-5 as w-scaled identity matmuls in PSUM + ACT stage (tap 6)
  batch 1: taps 0-4 on PE + ACT stage (tap 6) + one fused DVE tap (5)
Evictions are fused DVE adds (out = psum + acc), so the ACT/DVE accumulator
is folded in for free and PE never waits on PSUM reuse.

Sharding: batch across the 8 cores (2 per core). Only global_mean needs an
AllReduce of a [1,1536] fp32 vector.

fp16 is safe here: top-7 global_mean gap is 1.5e-3 while the fp16-input plus
fp16-S-eviction error is <6e-4 (validated against the fp32 FFT reference on
the actual seed); output tolerance is 2e-2 vs our ~7e-4.
"""

import numpy as np

B, L, H, E = 16, 1536, 8, 64
C = H * E             # 512 channels = H*E
NCORES = 8
BLOC = B // NCORES    # batches per core
NCC = C // 128        # channel chunks of 128
TOPK = 7              # int(1 * log(1536)) == 7
NJ = L // 128         # s-chunks
NLT = L // 512        # output l-tiles

PE_TAPS = (0, 1, 2, 3, 4, 5)   # batch 0 PE taps; batch 1 uses 0-4
B1_DVE_TAP = 5                 # batch 1 only: fused DVE tap
ACT_TAP = 6                    # staged by the ACT engine (both batches)
N_WARM = 8                     # PE warmup matmuls (p-state ramp during DMAs)
ECH = 896                      # first eviction/diag column chunk split

_cache = {}
DEBUG_BUILD = False


def _build(num_cores: int):
    import concourse.bass as bass
    import concourse.bacc as bacc
    import concourse.mybir as mybir
    import concourse.tile as tile

    f16 = mybir.dt.float16
    f32 = mybir.dt.float32
    u32 = mybir.dt.uint32
    PE = mybir.EngineType.PE
    ACT = mybir.EngineType.Activation
    DVE = mybir.EngineType.DVE
    MUL = mybir.AluOpType.mult
    ADD = mybir.AluOpType.add

    nc = bacc.Bacc(None)
    qT = nc.dram_tensor("qT", [BLOC, C, L], f16, kind="ExternalInput")
    kT = nc.dram_tensor("kT", [BLOC, C, L], f16, kind="ExternalInput")
    vT = nc.dram_tensor("vT", [BLOC, C, L], f16, kind="ExternalInput")
    out = nc.dram_tensor("out", [BLOC, C, L], f16, kind="ExternalOutput")
    if DEBUG_BUILD:
        dbg_gm = nc.dram_tensor("dbg_gm", [1, L], f32, kind="ExternalOutput")
        dbg_idx = nc.dram_tensor("dbg_idx", [1, 8], u32, kind="ExternalOutput")
        dbg_wr = nc.dram_tensor("dbg_wr", [BLOC, 8], f32, kind="ExternalOutput")
        dbg_mc = nc.dram_tensor("dbg_mc", [BLOC, L], f32, kind="ExternalOutput")
    ident_d = nc.inline_tensor(np.eye(128, dtype=np.float16), "identc")
    onesb_d = nc.inline_tensor(np.ones((1, 128), np.float16), "onesb")
    onesr_d = nc.inline_tensor(np.ones((128, 1), np.float16), "onesr")

    LD = L + 128          # doubled-tail S tile free size

    with tile.TileContext(nc) as tc:
        with (
            tc.tile_pool(name="sb", bufs=1) as sb,
            tc.tile_pool(name="sps", bufs=1, space="PSUM") as sps,
            tc.tile_pool(name="obp", bufs=3) as obp,
            tc.tile_pool(name="dram", bufs=1, space="DRAM") as dram,
        ):
            # ---- input loads: k/q first (Gram-critical), k on the SP
            # HWDGE queue, q + ident on the ACT queue; v afterwards ----
            ks = [[None] * NCC for _ in range(BLOC)]
            qs = [[None] * NCC for _ in range(BLOC)]
            ident = sb.tile([128, 128], f16, tag="ident")
            nc.sync.dma_start(ident, ident_d[:])
            q00 = sb.tile([128, L], f16, tag="q00")
            nc.scalar.dma_start(q00, qT[0, 0:128, :])
            qs[0][0] = q00
            k00 = sb.tile([128, L], f16, tag="k00")
            nc.sync.dma_start(k00, kT[0, 0:128, :])
            ks[0][0] = k00
            for bi in range(BLOC):
                for cc in range(NCC):
                    if bi == 0 and cc == 0:
                        continue
                    kt = sb.tile([128, L], f16, tag=f"k{bi}{cc}")
                    nc.sync.dma_start(kt, kT[bi, 128 * cc:128 * (cc + 1), :])
                    ks[bi][cc] = kt
                    qt = sb.tile([128, L], f16, tag=f"q{bi}{cc}")
                    nc.scalar.dma_start(qt, qT[bi, 128 * cc:128 * (cc + 1), :])
                    qs[bi][cc] = qt
                if bi == 0:
                    onesb = sb.tile([1, 128], f16, tag="onesb")
                    nc.sync.dma_start(onesb, onesb_d[:])
                    onesr = sb.tile([128, 1], f16, tag="onesr")
                    nc.sync.dma_start(onesr, onesr_d[:])

            # PE warmup: ramp the p-state while input DMAs stream (own PSUM
            # tag -- the wps tag is reused by the weight broadcasts later)
            warm = sps.tile([128, 128], f32, tag="wrm")
            for _ in range(N_WARM):
                nc.tensor.matmul(warm, ident, ident, start=True, stop=True,
                                 skip_group_check=True)

            # doubled v buffers: layout [128, NCC, 2L] so one dynamic-offset
            # AP can window all channel chunks at once
            vv = []
            for bi in range(BLOC):
                t = sb.tile([128, NCC, 2 * L], f16, tag=f"vv{bi}")
                for cc in range(NCC):
                    eng = nc.sync if cc % 2 == 0 else nc.scalar
                    eng.dma_start(t[:, cc, 0:L], vT[bi, 128 * cc:128 * (cc + 1), :])
                nc.vector.tensor_copy(t[:, :, L:2 * L], t[:, :, 0:L])
                vv.append(t)

            # ---- Gram with rotated PSUM accumulation; cc outermost so one
            # (k, q) chunk pair covers 12 s-blocks of matmul ----
            usegs = []
            for u in range(NJ):
                r = (L - 128 * u) % L
                segs = []
                t0 = 0
                while t0 < L:
                    y0 = (t0 + r) % L
                    seg = min(512 - (y0 % 512), L - t0, L - y0)
                    segs.append((t0, y0, seg))
                    t0 += seg
                usegs.append(segs)

            def gram_matmuls(bi, S, cc_range):
                for cc in cc_range:
                    for u in range(NJ):
                        for (ts_, ys_, seg) in usegs[u]:
                            nc.tensor.matmul(
                                S[:, ys_:ys_ + seg],
                                ks[bi][cc][:, 128 * u:128 * (u + 1)],
                                qs[bi][cc][:, ts_:ts_ + seg],
                                start=(u == 0 and cc == 0),
                                stop=(u == NJ - 1 and cc == NCC - 1),
                                skip_group_check=True,
                            )

            def evict_sdb(bi, S):
                # PSUM -> fp16 SBUF with the 1/HE scale fused, split across
                # ACT and DVE so both halves run concurrently; the wrapped
                # tail copy duplicates cols [0,128) at [L,L+128) so the
                # diagonal is one rectangle. The per-partition +1 element
                # skew is NOT expressible for compute engines or (in the
                # compiled DMA lowering) SBUF-side APs, so realign via DRAM:
                # write row p at flat offset (LD-1)*p (skewed -p per row),
                # read back with row stride LD -> diag[p, j] = sdb[p, p+j].
                # Rows overlap at one never-read address per pair (p<=127
                # keeps reads clear of it).
                sdb = sb.tile([128, LD], f16, tag=f"sdb{bi}")
                nc.scalar.mul(sdb[:, 0:768], S[:, 0:768], 1.0 / C)
                nc.vector.tensor_scalar_mul(sdb[:, 768:L], S[:, 768:L], 1.0 / C)
                nc.vector.tensor_copy(sdb[:, L:LD], sdb[:, 0:128])
                skew = dram.tile([128, LD], f16)
                sk = skew[:]
                nc.sync.dma_start(
                    bass.AP(sk.tensor, sk.offset, [(LD - 1, 128), (1, LD)]),
                    sdb[:])
                dg = sb.tile([128, L], f16, tag=f"diag{bi}")
                nc.sync.dma_start(
                    dg[:],
                    bass.AP(sk.tensor, sk.offset, [(LD, 128), (1, L)]))

                def diag(ch):
                    return dg[:, 512 * ch:512 * (ch + 1)]
                return diag

            S0 = sps.tile([128, L], f32, tag="S0")
            gram_matmuls(0, S0, range(NCC))
            diag0 = evict_sdb(0, S0)

            S1 = sps.tile([128, L], f32, tag="S1")
            gram_matmuls(1, S1, range(0, 2))

            # mc0 = per-partition-reduced diag0, on the PE mid-Gram1
            # (S0's banks are free once its eviction ran)
            S0b = sps.tile([128, L], f32, tag="S0")
            for ch in range(NLT):
                nc.tensor.matmul(S0b[0:1, 512 * ch:512 * (ch + 1)], onesr,
                                 diag0(ch),
                                 start=True, stop=True, skip_group_check=True)
            mc0 = sb.tile([1, L], f32, tag="mc0")
            nc.scalar.copy(mc0, S0b[0:1, :])

            gram_matmuls(1, S1, range(2, NCC))
            diag1 = evict_sdb(1, S1)

            # gl = sum_p (diag0 + diag1): one PSUM accumulation group
            S1b = sps.tile([128, L], f32, tag="S1")
            for di, dg in enumerate((diag0, diag1)):
                for ch in range(NLT):
                    nc.tensor.matmul(S1b[0:1, 512 * ch:512 * (ch + 1)], onesr,
                                     dg(ch),
                                     start=(di == 0), stop=(di == 1),
                                     skip_group_check=True)

            S0c = sps.tile([128, L], f32, tag="S0")
            for ch in range(NLT):
                nc.tensor.matmul(S0c[0:1, 512 * ch:512 * (ch + 1)], onesr,
                                 diag1(ch),
                                 start=True, stop=True, skip_group_check=True)
            mc1 = sb.tile([1, L], f32, tag="mc1")
            nc.scalar.copy(mc1, S0c[0:1, :])

            # ---- global mean: cross-core AllReduce of gl ----
            if num_cores > 1:
                gl = sb.tile([1, L], f32, tag="gl")
                nc.scalar.copy(gl, S1b[0:1, :])
                cc_in = dram.tile([1, L], f32)
                cc_out = dram.tile([1, L], f32)
                nc.sync.dma_start(cc_in, gl)
                nc.gpsimd.collective_compute(
                    "AllReduce",
                    mybir.AluOpType.add,
                    replica_groups=[list(range(num_cores))],
                    ins=[cc_in.opt()],
                    outs=[cc_out.opt()],
                )
                gm = sb.tile([1, L], f32, tag="gm")
                nc.sync.dma_start(gm, cc_out)
            else:
                # single core: top-8 reads the PSUM accumulator directly
                gm = S1b[0:1, :]

            # ---- top-7 lags (top-8 instruction, first 7 used) ----
            vals = sb.tile([1, 8], f32, tag="vals")
            idxs = sb.tile([1, 8], u32, tag="idxs")
            nc.vector.max(vals, gm)
            nc.vector.max_index(idxs, vals, gm)
            if DEBUG_BUILD:
                nc.sync.dma_start(dbg_gm[:], gm)
                nc.sync.dma_start(dbg_idx[:], idxs)
                nc.sync.dma_start(dbg_mc[0:1, :], mc0)
                nc.sync.dma_start(dbg_mc[1:2, :], mc1)

            act_eng = nc.engines[ACT]
            dve_eng = nc.engines[DVE]
            pe_eng = nc.engines[PE]

            # PE tap offsets: x = (L - lag_k) + 512*lt
            sv_x = {}
            for k in PE_TAPS:
                rp = pe_eng.alloc_register(f"ip{k}")
                pe_eng.reg_load(rp, idxs[0:1, k:k + 1])
                ro = pe_eng.alloc_register(f"io{k}")
                pe_eng.reg_alu(ro, L, rp, mybir.AluOpType.subtract)
                for lt in range(NLT):
                    rx = pe_eng.alloc_register(f"ix{k}_{lt}")
                    pe_eng.reg_alu(rx, ro, 512 * lt, mybir.AluOpType.add)
                    sv_x[(k, lt)] = pe_eng.snap(rx, donate=True, min_val=1,
                                                max_val=L + 1024)
                pe_eng.free_register(ro)

            # DVE/ACT tap window offsets o = L - lag_k
            rp = dve_eng.alloc_register("dp5")
            dve_eng.reg_load(rp, idxs[0:1, B1_DVE_TAP:B1_DVE_TAP + 1])
            ro = dve_eng.alloc_register("do5")
            dve_eng.reg_alu(ro, L, rp, mybir.AluOpType.subtract)
            sv_dve = dve_eng.snap(ro, donate=True, min_val=1, max_val=L)
            rp = act_eng.alloc_register("ap6")
            act_eng.reg_load(rp, idxs[0:1, ACT_TAP:ACT_TAP + 1])
            ro = act_eng.alloc_register("ao6")
            act_eng.reg_alu(ro, L, rp, mybir.AluOpType.subtract)
            sv_act = act_eng.snap(ro, donate=True, min_val=1, max_val=L)

            # ---- weights: gather mean_corr[b, lag_k] straight from SBUF ----
            wr = []
            for bi in range(BLOC):
                wrt = sb.tile([1, 8], f32, tag=f"wr{bi}")
                wr.append(wrt)
            for k in range(TOPK):
                rg = dve_eng.alloc_register(f"dg{k}")
                dve_eng.reg_load(rg, idxs[0:1, k:k + 1])
                sv = dve_eng.snap(rg, donate=True, min_val=0, max_val=L - 1)
                nc.vector.tensor_copy(wr[0][0:1, k:k + 1],
                                      mc0[0:1, bass.ds(sv, 1)])
            for k in range(TOPK):
                rg = act_eng.alloc_register(f"ag{k}")
                act_eng.reg_load(rg, idxs[0:1, k:k + 1])
                sv = act_eng.snap(rg, donate=True, min_val=0, max_val=L - 1)
                nc.scalar.copy(wr[1][0:1, k:k + 1], mc1[0:1, bass.ds(sv, 1)])

            if DEBUG_BUILD:
                nc.sync.dma_start(dbg_wr[0:1, :], wr[0])
                nc.sync.dma_start(dbg_wr[1:2, :], wr[1])

            # softmax per batch (no max-subtraction: |mean_corr| is small and
            # fp32 exp is safe; softmax is shift-invariant)
            wbcs = []
            for bi in range(BLOC):
                ex = sb.tile([1, 8], f32, tag=f"ex{bi}")
                nc.scalar.activation(ex[0:1, 0:TOPK], wr[bi][0:1, 0:TOPK],
                                     mybir.ActivationFunctionType.Exp,
                                     bias=0.0, scale=1.0)
                sm = sb.tile([1, 1], f32, tag=f"sm{bi}")
                nc.vector.reduce_sum(sm, ex[0:1, 0:TOPK],
                                     axis=mybir.AxisListType.X)
                rs = sb.tile([1, 1], f32, tag=f"rs{bi}")
                nc.vector.reciprocal(rs, sm)
                w16 = sb.tile([1, 8], f16, tag=f"w16{bi}")
                nc.scalar.mul(w16[0:1, 0:TOPK], ex[0:1, 0:TOPK], rs[0:1, 0:1])
                # broadcast to 128 partitions via ones-column matmul
                wps = sps.tile([128, 128], f32, tag="wps")
                nc.tensor.matmul(wps[:, 0:TOPK], onesb, w16[0:1, 0:TOPK],
                                 start=True, stop=True, skip_group_check=True)
                wbc = sb.tile([128, 8], f32, tag=f"wbc{bi}")
                nc.scalar.copy(wbc[:, 0:TOPK], wps[:, 0:TOPK])
                wbcs.append(wbc)

            # PE p-state filler: idle gaps reset the Tensor engine to a 2x
            # slower state for its next 3us of work; a run of tiny identity
            # matmuls bridges the topk/softmax wait so the output taps start
            # at full speed
            warm2 = sps.tile([128, 128], f32, tag="wrm")
            for _ in range(24):
                nc.tensor.matmul(warm2, ident, ident, start=True, stop=True,
                                 skip_group_check=True)

            # w-scaled identities for the PE taps
            Iw = [[None] * TOPK for _ in range(BLOC)]
            for bi in range(BLOC):
                taps = PE_TAPS if bi == 0 else PE_TAPS[:-1]
                for k in taps:
                    t = sb.tile([128, 128], f16, tag=f"iw{bi}{k}")
                    nc.vector.tensor_scalar_mul(t, ident, wbcs[bi][:, k:k + 1])
                    Iw[bi][k] = t

            # ---- weighted circular gather-sum ----
            # acc chains, chunked per channel chunk so cc0's eviction never
            # waits for later chunks: b0 = ACT stage only; b1 = ACT stage +
            # fused DVE tap (DVE chunks interleave behind b0's evictions)
            acc0 = sb.tile([128, NCC, L], f16, tag="acc0")
            for cc in range(NCC):
                nc.scalar.mul(acc0[:, cc, :],
                              vv[0][:, cc, bass.ds(sv_act, L)],
                              wbcs[0][:, ACT_TAP:ACT_TAP + 1])
            acc1s = sb.tile([128, NCC, L], f16, tag="acc1s")
            for cc in range(NCC):
                nc.scalar.mul(acc1s[:, cc, :],
                              vv[1][:, cc, bass.ds(sv_act, L)],
                              wbcs[1][:, ACT_TAP:ACT_TAP + 1])
            # b1's seventh tap: fused DVE chunks, interleaved into the
            # eviction stream below so b0's evictions are never blocked
            acc1 = sb.tile([128, NCC, L], f16, tag="acc1")
            accs = [acc0, acc1]

            # PE taps into PSUM; fused DVE eviction adds the chain acc
            for bi in range(BLOC):
                taps = PE_TAPS if bi == 0 else PE_TAPS[:-1]
                for cc in range(NCC):
                    tgt = sps.tile([128, L], f32,
                                   tag=("S0" if cc % 2 == 0 else "S1"))
                    ot = obp.tile([128, L], f16, tag="ot")
                    for lt in range(NLT):
                        for ki, k in enumerate(taps):
                            nc.tensor.matmul(
                                tgt[:, 512 * lt:512 * (lt + 1)],
                                Iw[bi][k],
                                vv[bi][:, cc, bass.ds(sv_x[(k, lt)], 512)],
                                start=(ki == 0),
                                stop=(ki == len(taps) - 1),
                                skip_group_check=True,
                            )
                        nc.vector.scalar_tensor_tensor(
                            ot[:, 512 * lt:512 * (lt + 1)],
                            tgt[:, 512 * lt:512 * (lt + 1)],
                            1.0,
                            accs[bi][:, cc, 512 * lt:512 * (lt + 1)],
                            op0=MUL, op1=ADD)
                        eng = nc.sync if (cc * NLT + lt) % 2 == 0 else nc.scalar
                        eng.dma_start(
                            out[bi, 128 * cc:128 * (cc + 1),
                                512 * lt:512 * (lt + 1)],
                            ot[:, 512 * lt:512 * (lt + 1)],
                        )
                    if bi == 0:
                        nc.vector.scalar_tensor_tensor(
                            acc1[:, cc, :],
                            vv[1][:, cc, bass.ds(sv_dve, L)],
                            wbcs[1][:, B1_DVE_TAP:B1_DVE_TAP + 1],
                            acc1s[:, cc, :],
                            op0=MUL, op1=ADD)
    nc.finalize()
    return nc


def _marshal(arr, ncores):
    # [B, L, H, E] fp32 -> per-core contiguous fp16 [BLOC, C, L]
    a = arr.reshape(B, L, C).astype(np.float16)
    a = np.ascontiguousarray(a.transpose(0, 2, 1))  # [B, C, L]
    bloc = B // ncores
    return [a[c * bloc:(c + 1) * bloc] for c in range(ncores)]


def _ensure_axon_hooks_importable():
    # some containers lack antenv.axon_hooks; run_bass_kernel_spmd imports it
    # unconditionally when tracing is requested. A None hook degrades to an
    # untraced run instead of crashing.
    import sys
    import types
    try:
        import antenv.axon_hooks  # noqa: F401
    except ModuleNotFoundError:
        try:
            import antenv
        except ModuleNotFoundError:
            return
        m = types.ModuleType("antenv.axon_hooks")
        m.get_axon_ntff_profile_hook = lambda: None
        sys.modules["antenv.axon_hooks"] = m
        antenv.axon_hooks = m


def kernel(queries, keys, values, attn_mask=None, _trace=False):
    from concourse.bass_utils import run_bass_kernel_spmd

    _ensure_axon_hooks_importable()

    nc = _cache.get("nc")
    if nc is None:
        nc = _build(NCORES)
        _cache["nc"] = nc

    qs = _marshal(np.asarray(queries, np.float32), NCORES)
    ks = _marshal(np.asarray(keys, np.float32), NCORES)
    vs = _marshal(np.asarray(values, np.float32), NCORES)
    in_maps = [{"qT": qs[c], "kT": ks[c], "vT": vs[c]} for c in range(NCORES)]

    res = run_bass_kernel_spmd(nc, in_maps, core_ids=list(range(NCORES)), trace=_trace)
    _cache["last"] = res
    o = np.concatenate([res.results[c]["out"] for c in range(NCORES)], axis=0)
    o = o.transpose(0, 2, 1).astype(np.float32)  # [B, L, C]
    return np.ascontiguousarray(o.reshape(B, L, H, E))


# revision 57
# speedup vs baseline: 1.4447x; 1.0203x over previous
"""AutoCorrelation (FFT-free) kernel for 8 Trainium2 NeuronCores.

Math: the reference computes, per (b, h, e), the circular cross-correlation
corr = irfft(rfft(q) * conj(rfft(k))), then
  mean_corr[b, l] = mean_{h,e} corr          (only this is ever used)
  global_mean[l]  = mean_b mean_corr
  topk lags       = top-7 of global_mean
  weights         = softmax(mean_corr[:, topk])
  out[b,l]        = sum_k w[b,k] * v[b, (l - lag_k) % L]

Identity used: mean_corr[b, l] = (1/HE) * sum_s <q[b,(s+l)%L,:,:], k[b,s,:,:]>.
Per batch the Gram matrix G[s,t] = sum_c kT[c,s] qT[c,t] runs on the
TensorEngine (fp16 inputs, fp32 PSUM accumulation), with each s-chunk's
output columns rotated in PSUM so that afterwards
mean_corr[l] = sum_p S[p, (l+p)%L]. The channel loop is outermost so the
first (k, q) chunk pair covers 12 s-blocks (~7us) of matmul per load.

The diagonal fold runs as: evict S to a doubled fp16 SBUF tile (1/HE scale
fused), a diagonal-access-pattern DMA (per-partition +1 element skew,
expressible because DMA APs are flat element strides), then a ones-vector
matmul reduces the 128 partitions in PSUM (512-col chunks reusing the Gram
banks). Eviction + diagonal DMA are split in two column chunks to pipeline.
global_mean accumulates both batches' diagonals into one PSUM group on the
otherwise-idle PE. This replaces the previous 7-level rotate-add fold tree
(~25us of serial DVE/DMA ops) with ~4us that mostly hides under the Gram.

Top-7 via the DVE max/max_index top-8 instruction. Weights: dynamic-offset
ACT/DVE element copies gather mean_corr[b, lag_k] straight from SBUF (no
SWDGE round trips), softmax per batch at partition 0 (no cross-partition
DMA hop).

Output gather-sum sum_k w_k v[(l-lag_k)%L], cost-model-balanced across
engines (PE identity-matmul tap = 2.56us, ACT stage = 5.3us, DVE fused
tap = 6.5us, DVE fused eviction slice = 0.66us):
  batch 0: taps 0-5 as w-scaled identity matmuls in PSUM + ACT stage (tap 6)
  batch 1: taps 0-4 on PE + ACT stage (tap 6) + one fused DVE tap (5)
Evictions are fused DVE adds (out = psum + acc), so the ACT/DVE accumulator
is folded in for free and PE never waits on PSUM reuse.

Sharding: batch across the 8 cores (2 per core). Only global_mean needs an
AllReduce of a [1,1536] fp32 vector.

fp16 is safe here: top-7 global_mean gap is 1.5e-3 while the fp16-input plus
fp16-S-eviction error is <6e-4 (validated against the fp32 FFT reference on
the actual seed); output tolerance is 2e-2 vs our ~7e-4.
"""

import numpy as np

B, L, H, E = 16, 1536, 8, 64
C = H * E             # 512 channels = H*E
NCORES = 8
BLOC = B // NCORES    # batches per core
NCC = C // 128        # channel chunks of 128
TOPK = 7              # int(1 * log(1536)) == 7
NJ = L // 128         # s-chunks
NLT = L // 512        # output l-tiles

PE_TAPS = (0, 1, 2, 3, 4, 5)   # batch 0 PE taps; batch 1 uses 0-4
B1_DVE_TAP = 5                 # batch 1 only: fused DVE tap
ACT_TAP = 6                    # staged by the ACT engine (both batches)
N_WARM = 48                    # PE warmup matmuls (p-state ramp during DMAs)
ECH = 896                      # first eviction/diag column chunk split

_cache = {}
DEBUG_BUILD = False


def _build(num_cores: int):
    import concourse.bass as bass
    import concourse.bacc as bacc
    import concourse.mybir as mybir
    import concourse.tile as tile

    f16 = mybir.dt.float16
    f32 = mybir.dt.float32
    u32 = mybir.dt.uint32
    PE = mybir.EngineType.PE
    ACT = mybir.EngineType.Activation
    DVE = mybir.EngineType.DVE
    MUL = mybir.AluOpType.mult
    ADD = mybir.AluOpType.add

    nc = bacc.Bacc(None)
    qT = nc.dram_tensor("qT", [BLOC, C, L], f16, kind="ExternalInput")
    kT = nc.dram_tensor("kT", [BLOC, C, L], f16, kind="ExternalInput")
    vT = nc.dram_tensor("vT", [BLOC, C, L], f16, kind="ExternalInput")
    out = nc.dram_tensor("out", [BLOC, C, L], f16, kind="ExternalOutput")
    if DEBUG_BUILD:
        dbg_gm = nc.dram_tensor("dbg_gm", [1, L], f32, kind="ExternalOutput")
        dbg_idx = nc.dram_tensor("dbg_idx", [1, 8], u32, kind="ExternalOutput")
        dbg_wr = nc.dram_tensor("dbg_wr", [BLOC, 8], f32, kind="ExternalOutput")
        dbg_mc = nc.dram_tensor("dbg_mc", [BLOC, L], f32, kind="ExternalOutput")
    ident_d = nc.inline_tensor(np.eye(128, dtype=np.float16), "identc")
    onesb_d = nc.inline_tensor(np.ones((1, 128), np.float16), "onesb")
    onesr_d = nc.inline_tensor(np.ones((128, 1), np.float16), "onesr")

    LD = L + 128          # doubled-tail S tile free size

    with tile.TileContext(nc) as tc:
        with (
            tc.tile_pool(name="sb", bufs=1) as sb,
            tc.tile_pool(name="sps", bufs=1, space="PSUM") as sps,
            tc.tile_pool(name="obp", bufs=3) as obp,
            tc.tile_pool(name="dram", bufs=1, space="DRAM") as dram,
        ):
            # ---- input loads: k/q first (Gram-critical) interleaved on
            # both HWDGE queues; the first matmul's operands (full q00 but
            # only the first 128 cols of k00) lead both queues ----
            ks = [[None] * NCC for _ in range(BLOC)]
            qs = [[None] * NCC for _ in range(BLOC)]
            k00 = sb.tile([128, L], f16, tag="k00")
            nc.sync.dma_start(k00, kT[0, 0:128, :])
            ks[0][0] = k00
            q00 = sb.tile([128, L], f16, tag="q00")
            nc.scalar.dma_start(q00, qT[0, 0:128, :])
            qs[0][0] = q00
            ident = sb.tile([128, 128], f16, tag="ident")
            nc.scalar.dma_start(ident, ident_d[:])
            for bi in range(BLOC):
                for cc in range(NCC):
                    if bi == 0 and cc == 0:
                        continue
                    kt = sb.tile([128, L], f16, tag=f"k{bi}{cc}")
                    nc.sync.dma_start(kt, kT[bi, 128 * cc:128 * (cc + 1), :])
                    ks[bi][cc] = kt
                    qt = sb.tile([128, L], f16, tag=f"q{bi}{cc}")
                    nc.scalar.dma_start(qt, qT[bi, 128 * cc:128 * (cc + 1), :])
                    qs[bi][cc] = qt
                if bi == 0:
                    onesb = sb.tile([1, 128], f16, tag="onesb")
                    nc.sync.dma_start(onesb, onesb_d[:])
                    onesr = sb.tile([128, 1], f16, tag="onesr")
                    nc.sync.dma_start(onesr, onesr_d[:])

            # PE warmup on a memset tile: long enough to run continuously
            # into the first Gram matmul so the p-state ramp completes before
            # real work starts (any idle gap resets the Tensor engine to a
            # 2x-slower state for its next 3us)
            junk = sb.tile([128, 128], f16, tag="junk")
            nc.vector.memset(junk, 0.0)
            warm = sps.tile([128, 128], f32, tag="wrm")
            for _ in range(N_WARM):
                nc.tensor.matmul(warm, junk, junk, start=True, stop=True,
                                 skip_group_check=True)

            # doubled v buffers: layout [128, NCC, 2L] so one dynamic-offset
            # AP can window all channel chunks at once
            vv = []
            for bi in range(BLOC):
                t = sb.tile([128, NCC, 2 * L], f16, tag=f"vv{bi}")
                for cc in range(NCC):
                    eng = nc.sync if cc % 2 == 0 else nc.scalar
                    eng.dma_start(t[:, cc, 0:L], vT[bi, 128 * cc:128 * (cc + 1), :])
                nc.vector.tensor_copy(t[:, :, L:2 * L], t[:, :, 0:L])
                vv.append(t)

            # ---- Gram with rotated PSUM accumulation; cc outermost so one
            # (k, q) chunk pair covers 12 s-blocks of matmul ----
            usegs = []
            for u in range(NJ):
                r = (L - 128 * u) % L
                segs = []
                t0 = 0
                while t0 < L:
                    y0 = (t0 + r) % L
                    seg = min(512 - (y0 % 512), L - t0, L - y0)
                    segs.append((t0, y0, seg))
                    t0 += seg
                usegs.append(segs)

            def gram_matmuls(bi, S, cc_range):
                for cc in cc_range:
                    for u in range(NJ):
                        for (ts_, ys_, seg) in usegs[u]:
                            nc.tensor.matmul(
                                S[:, ys_:ys_ + seg],
                                ks[bi][cc][:, 128 * u:128 * (u + 1)],
                                qs[bi][cc][:, ts_:ts_ + seg],
                                start=(u == 0 and cc == 0),
                                stop=(u == NJ - 1 and cc == NCC - 1),
                                skip_group_check=True,
                            )

            def evict_sdb(bi, S):
                # PSUM -> fp16 SBUF with the 1/HE scale fused, split across
                # ACT and DVE so both halves run concurrently; the wrapped
                # tail copy duplicates cols [0,128) at [L,L+128) so the
                # diagonal is one rectangle. The per-partition +1 element
                # skew is NOT expressible for compute engines or (in the
                # compiled DMA lowering) SBUF-side APs, so realign via DRAM:
                # write row p at flat offset (LD-1)*p (skewed -p per row),
                # read back with row stride LD -> diag[p, j] = sdb[p, p+j].
                # Rows overlap at one never-read address per pair (p<=127
                # keeps reads clear of it).
                sdb = sb.tile([128, LD], f16, tag=f"sdb{bi}")
                nc.scalar.mul(sdb[:, 0:L], S, 1.0 / C)
                nc.scalar.copy(sdb[:, L:LD], sdb[:, 0:128])
                # two independent DRAM staging tensors so the second write
                # chunk and first read-back chunk pipeline (a single tensor
                # would serialize on conservative whole-tile deps)
                skewA = dram.tile([128, LD], f16)
                skewB = dram.tile([128, LD], f16)
                ska, skb = skewA[:], skewB[:]
                nc.sync.dma_start(
                    bass.AP(ska.tensor, ska.offset, [(LD - 1, 128), (1, 896)]),
                    sdb[:, 0:896])
                nc.scalar.dma_start(
                    bass.AP(skb.tensor, skb.offset, [(LD - 1, 128), (1, LD - 768)]),
                    sdb[:, 768:LD])
                dg = sb.tile([128, L], f16, tag=f"diag{bi}")
                nc.sync.dma_start(
                    dg[:, 0:768],
                    bass.AP(ska.tensor, ska.offset, [(LD, 128), (1, 768)]))
                nc.scalar.dma_start(
                    dg[:, 768:L],
                    bass.AP(skb.tensor, skb.offset, [(LD, 128), (1, 768)]))

                def diag(ch):
                    return dg[:, 512 * ch:512 * (ch + 1)]
                return diag

            S0 = sps.tile([128, L], f32, tag="S0")
            gram_matmuls(0, S0, range(NCC))
            diag0 = evict_sdb(0, S0)

            S1 = sps.tile([128, L], f32, tag="S1")
            gram_matmuls(1, S1, range(0, 2))

            # mc0 = per-partition-reduced diag0, on the PE mid-Gram1
            # (S0's banks are free once its eviction ran)
            S0b = sps.tile([128, L], f32, tag="S0")
            for ch in range(NLT):
                nc.tensor.matmul(S0b[0:1, 512 * ch:512 * (ch + 1)], onesr,
                                 diag0(ch),
                                 start=True, stop=True, skip_group_check=True)
            mc0 = sb.tile([1, L], f32, tag="mc0")
            nc.scalar.copy(mc0, S0b[0:1, :])

            gram_matmuls(1, S1, range(2, NCC))
            diag1 = evict_sdb(1, S1)

            # gl = sum_p (diag0 + diag1): one PSUM accumulation group
            S1b = sps.tile([128, L], f32, tag="S1")
            for di, dg in enumerate((diag0, diag1)):
                for ch in range(NLT):
                    nc.tensor.matmul(S1b[0:1, 512 * ch:512 * (ch + 1)], onesr,
                                     dg(ch),
                                     start=(di == 0), stop=(di == 1),
                                     skip_group_check=True)

            S0c = sps.tile([128, L], f32, tag="S0")
            for ch in range(NLT):
                nc.tensor.matmul(S0c[0:1, 512 * ch:512 * (ch + 1)], onesr,
                                 diag1(ch),
                                 start=True, stop=True, skip_group_check=True)
            mc1 = sb.tile([1, L], f32, tag="mc1")
            nc.scalar.copy(mc1, S0c[0:1, :])

            # ---- global mean: cross-core AllReduce of gl ----
            if num_cores > 1:
                gl = sb.tile([1, L], f32, tag="gl")
                nc.scalar.copy(gl, S1b[0:1, :])
                cc_in = dram.tile([1, L], f32)
                cc_out = dram.tile([1, L], f32)
                nc.sync.dma_start(cc_in, gl)
                nc.gpsimd.collective_compute(
                    "AllReduce",
                    mybir.AluOpType.add,
                    replica_groups=[list(range(num_cores))],
                    ins=[cc_in.opt()],
                    outs=[cc_out.opt()],
                )
                gm = sb.tile([1, L], f32, tag="gm")
                nc.sync.dma_start(gm, cc_out)
            else:
                # single core: top-8 reads the PSUM accumulator directly
                gm = S1b[0:1, :]

            # ---- top-7 lags (top-8 instruction, first 7 used) ----
            vals = sb.tile([1, 8], f32, tag="vals")
            idxs = sb.tile([1, 8], u32, tag="idxs")
            nc.vector.max(vals, gm)
            nc.vector.max_index(idxs, vals, gm)
            if DEBUG_BUILD:
                nc.sync.dma_start(dbg_gm[:], gm)
                nc.sync.dma_start(dbg_idx[:], idxs)
                nc.sync.dma_start(dbg_mc[0:1, :], mc0)
                nc.sync.dma_start(dbg_mc[1:2, :], mc1)

            act_eng = nc.engines[ACT]
            dve_eng = nc.engines[DVE]
            pe_eng = nc.engines[PE]

            # PE tap offsets: x = (L - lag_k) + 512*lt
            sv_x = {}
            for k in PE_TAPS:
                rp = pe_eng.alloc_register(f"ip{k}")
                pe_eng.reg_load(rp, idxs[0:1, k:k + 1])
                ro = pe_eng.alloc_register(f"io{k}")
                pe_eng.reg_alu(ro, L, rp, mybir.AluOpType.subtract)
                for lt in range(NLT):
                    rx = pe_eng.alloc_register(f"ix{k}_{lt}")
                    pe_eng.reg_alu(rx, ro, 512 * lt, mybir.AluOpType.add)
                    sv_x[(k, lt)] = pe_eng.snap(rx, donate=True, min_val=1,
                                                max_val=L + 1024)
                pe_eng.free_register(ro)

            # DVE/ACT tap window offsets o = L - lag_k
            rp = dve_eng.alloc_register("dp5")
            dve_eng.reg_load(rp, idxs[0:1, B1_DVE_TAP:B1_DVE_TAP + 1])
            ro = dve_eng.alloc_register("do5")
            dve_eng.reg_alu(ro, L, rp, mybir.AluOpType.subtract)
            sv_dve_lt = []
            for lt in range(NLT):
                rx = dve_eng.alloc_register(f"dx{lt}")
                dve_eng.reg_alu(rx, ro, 512 * lt, mybir.AluOpType.add)
                sv_dve_lt.append(dve_eng.snap(rx, donate=True, min_val=1,
                                              max_val=L + 512 * lt))
            sv_dve = dve_eng.snap(ro, donate=True, min_val=1, max_val=L)
            rp = act_eng.alloc_register("ap6")
            act_eng.reg_load(rp, idxs[0:1, ACT_TAP:ACT_TAP + 1])
            ro = act_eng.alloc_register("ao6")
            act_eng.reg_alu(ro, L, rp, mybir.AluOpType.subtract)
            sv_act = act_eng.snap(ro, donate=True, min_val=1, max_val=L)

            # ---- weights: gather mean_corr[b, lag_k] straight from SBUF ----
            wr = []
            for bi in range(BLOC):
                wrt = sb.tile([1, 8], f32, tag=f"wr{bi}")
                wr.append(wrt)
            for k in range(TOPK):
                rg = dve_eng.alloc_register(f"dg{k}")
                dve_eng.reg_load(rg, idxs[0:1, k:k + 1])
                sv = dve_eng.snap(rg, donate=True, min_val=0, max_val=L - 1)
                nc.vector.tensor_copy(wr[0][0:1, k:k + 1],
                                      mc0[0:1, bass.ds(sv, 1)])
            for k in range(TOPK):
                rg = act_eng.alloc_register(f"ag{k}")
                act_eng.reg_load(rg, idxs[0:1, k:k + 1])
                sv = act_eng.snap(rg, donate=True, min_val=0, max_val=L - 1)
                nc.scalar.copy(wr[1][0:1, k:k + 1], mc1[0:1, bass.ds(sv, 1)])

            if DEBUG_BUILD:
                nc.sync.dma_start(dbg_wr[0:1, :], wr[0])
                nc.sync.dma_start(dbg_wr[1:2, :], wr[1])

            # softmax per batch (no max-subtraction: |mean_corr| is small and
            # fp32 exp is safe; softmax is shift-invariant)
            wbcs = []
            for bi in range(BLOC):
                ex = sb.tile([1, 8], f32, tag=f"ex{bi}")
                nc.scalar.activation(ex[0:1, 0:TOPK], wr[bi][0:1, 0:TOPK],
                                     mybir.ActivationFunctionType.Exp,
                                     bias=0.0, scale=1.0)
                sm = sb.tile([1, 1], f32, tag=f"sm{bi}")
                nc.vector.reduce_sum(sm, ex[0:1, 0:TOPK],
                                     axis=mybir.AxisListType.X)
                rs = sb.tile([1, 1], f32, tag=f"rs{bi}")
                nc.vector.reciprocal(rs, sm)
                w16 = sb.tile([1, 8], f16, tag=f"w16{bi}")
                nc.vector.tensor_scalar_mul(w16[0:1, 0:TOPK], ex[0:1, 0:TOPK],
                                            rs[0:1, 0:1])
                # broadcast to 128 partitions via ones-column matmul
                wps = sps.tile([128, 128], f32, tag="wps")
                nc.tensor.matmul(wps[:, 0:TOPK], onesb, w16[0:1, 0:TOPK],
                                 start=True, stop=True, skip_group_check=True)
                wbc = sb.tile([128, 8], f32, tag=f"wbc{bi}")
                nc.scalar.copy(wbc[:, 0:TOPK], wps[:, 0:TOPK])
                wbcs.append(wbc)

            # w-scaled identities for the PE taps
            Iw = [[None] * TOPK for _ in range(BLOC)]
            for bi in range(BLOC):
                taps = PE_TAPS if bi == 0 else PE_TAPS[:-1]
                for k in taps:
                    t = sb.tile([128, 128], f16, tag=f"iw{bi}{k}")
                    nc.vector.tensor_scalar_mul(t, ident, wbcs[bi][:, k:k + 1])
                    Iw[bi][k] = t

            # ---- weighted circular gather-sum ----
            # acc chains, chunked per channel chunk so cc0's eviction never
            # waits for later chunks: b0 = ACT stage only; b1 = ACT stage +
            # fused DVE tap (DVE chunks interleave behind b0's evictions)
            acc0 = sb.tile([128, NCC, L], f16, tag="acc0")
            for cc in range(NCC):
                nc.scalar.mul(acc0[:, cc, :],
                              vv[0][:, cc, bass.ds(sv_act, L)],
                              wbcs[0][:, ACT_TAP:ACT_TAP + 1])
            acc1s = sb.tile([128, NCC, L], f16, tag="acc1s")
            for cc in range(NCC):
                nc.scalar.mul(acc1s[:, cc, :],
                              vv[1][:, cc, bass.ds(sv_act, L)],
                              wbcs[1][:, ACT_TAP:ACT_TAP + 1])
            acc1 = sb.tile([128, NCC, L], f16, tag="acc1")
            accs = [acc0, acc1]

            # PE taps into PSUM; fused DVE eviction adds the chain acc;
            # b1's fused DVE tap chunks ride behind b0's evictions
            for bi in range(BLOC):
                taps = PE_TAPS if bi == 0 else PE_TAPS[:-1]
                for cc in range(NCC):
                    tgt = sps.tile([128, L], f32,
                                   tag=("S0" if cc % 2 == 0 else "S1"))
                    ot = obp.tile([128, L], f16, tag="ot")
                    fine = (bi == 1 and cc == NCC - 1)
                    for lt in range(NLT):
                        for ki, k in enumerate(taps):
                            nc.tensor.matmul(
                                tgt[:, 512 * lt:512 * (lt + 1)],
                                Iw[bi][k],
                                vv[bi][:, cc, bass.ds(sv_x[(k, lt)], 512)],
                                start=(ki == 0),
                                stop=(ki == len(taps) - 1),
                                skip_group_check=True,
                            )
                        if fine:
                            nc.vector.scalar_tensor_tensor(
                                ot[:, 512 * lt:512 * (lt + 1)],
                                tgt[:, 512 * lt:512 * (lt + 1)],
                                1.0,
                                accs[bi][:, cc, 512 * lt:512 * (lt + 1)],
                                op0=MUL, op1=ADD)
                            eng = nc.sync if lt % 2 == 0 else nc.scalar
                            eng.dma_start(
                                out[bi, 128 * cc:128 * (cc + 1),
                                    512 * lt:512 * (lt + 1)],
                                ot[:, 512 * lt:512 * (lt + 1)],
                            )
                    if bi == 0:
                        nc.vector.scalar_tensor_tensor(
                            acc1[:, cc, :],
                            vv[1][:, cc, bass.ds(sv_dve, L)],
                            wbcs[1][:, B1_DVE_TAP:B1_DVE_TAP + 1],
                            acc1s[:, cc, :],
                            op0=MUL, op1=ADD)
                    if not fine:
                        # one coarse eviction per channel chunk: fewer DVE
                        # ops (DVE paces the output phase); stores stay
                        # sliced for DMA streaming
                        nc.vector.scalar_tensor_tensor(
                            ot, tgt, 1.0, accs[bi][:, cc, :],
                            op0=MUL, op1=ADD)
                        for lt in range(NLT):
                            eng = nc.sync if (cc * NLT + lt) % 2 == 0 else nc.scalar
                            eng.dma_start(
                                out[bi, 128 * cc:128 * (cc + 1),
                                    512 * lt:512 * (lt + 1)],
                                ot[:, 512 * lt:512 * (lt + 1)],
                            )
    nc.finalize()
    return nc


def _marshal(arr, ncores):
    # [B, L, H, E] fp32 -> per-core contiguous fp16 [BLOC, C, L]
    a = arr.reshape(B, L, C).astype(np.float16)
    a = np.ascontiguousarray(a.transpose(0, 2, 1))  # [B, C, L]
    bloc = B // ncores
    return [a[c * bloc:(c + 1) * bloc] for c in range(ncores)]


def _ensure_axon_hooks_importable():
    # some containers lack antenv.axon_hooks; run_bass_kernel_spmd imports it
    # unconditionally when tracing is requested. A None hook degrades to an
    # untraced run instead of crashing.
    import sys
    import types
    try:
        import antenv.axon_hooks  # noqa: F401
    except ModuleNotFoundError:
        try:
            import antenv
        except ModuleNotFoundError:
            return
        m = types.ModuleType("antenv.axon_hooks")
        m.get_axon_ntff_profile_hook = lambda: None
        sys.modules["antenv.axon_hooks"] = m
        antenv.axon_hooks = m


def kernel(queries, keys, values, attn_mask=None, _trace=False):
    from concourse.bass_utils import run_bass_kernel_spmd

    _ensure_axon_hooks_importable()

    nc = _cache.get("nc")
    if nc is None:
        nc = _build(NCORES)
        _cache["nc"] = nc

    qs = _marshal(np.asarray(queries, np.float32), NCORES)
    ks = _marshal(np.asarray(keys, np.float32), NCORES)
    vs = _marshal(np.asarray(values, np.float32), NCORES)
    in_maps = [{"qT": qs[c], "kT": ks[c], "vT": vs[c]} for c in range(NCORES)]

    res = run_bass_kernel_spmd(nc, in_maps, core_ids=list(range(NCORES)), trace=_trace)
    _cache["last"] = res
    o = np.concatenate([res.results[c]["out"] for c in range(NCORES)], axis=0)
    o = o.transpose(0, 2, 1).astype(np.float32)  # [B, L, C]
    return np.ascontiguousarray(o.reshape(B, L, H, E))


# revision 70
# speedup vs baseline: 1.4847x; 1.0277x over previous
"""AutoCorrelation (FFT-free) kernel for 8 Trainium2 NeuronCores.

Math: the reference computes, per (b, h, e), the circular cross-correlation
corr = irfft(rfft(q) * conj(rfft(k))), then
  mean_corr[b, l] = mean_{h,e} corr          (only this is ever used)
  global_mean[l]  = mean_b mean_corr
  topk lags       = top-7 of global_mean
  weights         = softmax(mean_corr[:, topk])
  out[b,l]        = sum_k w[b,k] * v[b, (l - lag_k) % L]

Identity used: mean_corr[b, l] = (1/HE) * sum_s <q[b,(s+l)%L,:,:], k[b,s,:,:]>.
Per batch the Gram matrix G[s,t] = sum_c kT[c,s] qT[c,t] runs on the
TensorEngine (fp16 inputs, fp32 PSUM accumulation), with each s-chunk's
output columns rotated in PSUM so that afterwards
mean_corr[l] = sum_p S[p, (l+p)%L]. The channel loop is outermost so the
first (k, q) chunk pair covers 12 s-blocks (~7us) of matmul per load.

The diagonal fold: evict S to a tail-doubled fp16 SBUF tile (1/HE scale
fused), then realign the per-partition skew through DRAM -- write row p at
flat offset (LD-1)*p (the skew is expressible on the flat-addressed DRAM
side of a DMA; neither compute engines nor the compiled SBUF DMA lowering
accept a partition step of pitch+1), read back with row stride LD so
diag[p, j] = S[p, (p+j) % L], split in two column chunks on the two HWDGE
queues to pipeline. A ones-vector matmul then reduces the 128 partitions in
PSUM (512-col chunks reusing the Gram banks); global_mean accumulates both
batches' diagonals into one PSUM group on the otherwise-idle PE, and the
single-core build's top-8 reads that PSUM row directly. Batch 0's entire
fold hides under batch 1's Gram; this replaces the previous 7-level
rotate-add fold tree (~25us of serial DVE/DMA ops) with ~7us of which only
batch 1's DMA round trip is exposed.

Top-7 via the DVE max/max_index top-8 instruction. Weights: dynamic-offset
ACT/DVE element copies gather mean_corr[b, lag_k] straight from SBUF (no
SWDGE round trips), softmax per batch at partition 0 (no cross-partition
DMA hop).

Output gather-sum sum_k w_k v[(l-lag_k)%L], cost-model-balanced across
engines (PE identity-matmul tap = 2.56us, ACT stage = 5.3us, DVE fused
tap = 6.5us, DVE fused eviction slice = 0.66us):
  batch 0: taps 0-5 as w-scaled identity matmuls in PSUM + ACT stage (tap 6)
  batch 1: taps 0-4 on PE + ACT stage (tap 6) + one fused DVE tap (5)
Evictions are fused DVE adds (out = psum + acc), so the ACT/DVE accumulator
is folded in for free and PE never waits on PSUM reuse.

Sharding: batch across the 8 cores (2 per core). Only global_mean needs an
AllReduce of a [1,1536] fp32 vector.

fp16 is safe here: top-7 global_mean gap is 1.5e-3 while the fp16-input plus
fp16-S-eviction error is <6e-4 (validated against the fp32 FFT reference on
the actual seed); output tolerance is 2e-2 vs our ~7e-4.
"""

import numpy as np

B, L, H, E = 16, 1536, 8, 64
C = H * E             # 512 channels = H*E
NCORES = 8
BLOC = B // NCORES    # batches per core
NCC = C // 128        # channel chunks of 128
TOPK = 7              # int(1 * log(1536)) == 7
NJ = L // 128         # s-chunks
NLT = L // 512        # output l-tiles

PE_TAPS = (0, 1, 2, 3, 4, 5)   # batch 0 PE taps; batch 1 uses 0-4
B1_DVE_TAP = 5                 # batch 1 only: fused DVE tap
ACT_TAP = 6                    # staged by the ACT engine (both batches)
N_WARM = 48                    # PE warmup matmuls (p-state ramp during DMAs)
ECH = 896                      # first eviction/diag column chunk split

_cache = {}
DEBUG_BUILD = False


def _build(num_cores: int):
    import concourse.bass as bass
    import concourse.bacc as bacc
    import concourse.mybir as mybir
    import concourse.tile as tile

    f16 = mybir.dt.float16
    f32 = mybir.dt.float32
    u32 = mybir.dt.uint32
    PE = mybir.EngineType.PE
    ACT = mybir.EngineType.Activation
    DVE = mybir.EngineType.DVE
    MUL = mybir.AluOpType.mult
    ADD = mybir.AluOpType.add

    nc = bacc.Bacc(None)
    qT = nc.dram_tensor("qT", [BLOC, C, L], f16, kind="ExternalInput")
    kT = nc.dram_tensor("kT", [BLOC, C, L], f16, kind="ExternalInput")
    vT = nc.dram_tensor("vT", [BLOC, C, L], f16, kind="ExternalInput")
    out = nc.dram_tensor("out", [BLOC, C, L], f16, kind="ExternalOutput")
    if DEBUG_BUILD:
        dbg_gm = nc.dram_tensor("dbg_gm", [1, L], f32, kind="ExternalOutput")
        dbg_idx = nc.dram_tensor("dbg_idx", [1, 8], u32, kind="ExternalOutput")
        dbg_wr = nc.dram_tensor("dbg_wr", [BLOC, 8], f32, kind="ExternalOutput")
        dbg_mc = nc.dram_tensor("dbg_mc", [BLOC, L], f32, kind="ExternalOutput")
    ident_d = nc.inline_tensor(np.eye(128, dtype=np.float16), "identc")
    onesb_d = nc.inline_tensor(np.ones((1, 128), np.float16), "onesb")
    onesr_d = nc.inline_tensor(np.ones((128, 1), np.float16), "onesr")

    LD = L + 128          # doubled-tail S tile free size

    with tile.TileContext(nc) as tc:
        with (
            tc.tile_pool(name="sb", bufs=1) as sb,
            tc.tile_pool(name="sps", bufs=1, space="PSUM") as sps,
            tc.tile_pool(name="obp", bufs=3) as obp,
            tc.tile_pool(name="dram", bufs=1, space="DRAM") as dram,
        ):
            # ---- input loads: k/q first (Gram-critical) interleaved on
            # both HWDGE queues; the first matmul's operands (full q00 but
            # only the first 128 cols of k00) lead both queues ----
            ks = [[None] * NCC for _ in range(BLOC)]
            qs = [[None] * NCC for _ in range(BLOC)]
            k00 = sb.tile([128, L], f16, tag="k00")
            nc.sync.dma_start(k00, kT[0, 0:128, :])
            ks[0][0] = k00
            q00 = sb.tile([128, L], f16, tag="q00")
            nc.scalar.dma_start(q00, qT[0, 0:128, :])
            qs[0][0] = q00
            ident = sb.tile([128, 128], f16, tag="ident")
            nc.scalar.dma_start(ident, ident_d[:])
            for bi in range(BLOC):
                for cc in range(NCC):
                    if bi == 0 and cc == 0:
                        continue
                    kt = sb.tile([128, L], f16, tag=f"k{bi}{cc}")
                    nc.sync.dma_start(kt, kT[bi, 128 * cc:128 * (cc + 1), :])
                    ks[bi][cc] = kt
                    qt = sb.tile([128, L], f16, tag=f"q{bi}{cc}")
                    nc.scalar.dma_start(qt, qT[bi, 128 * cc:128 * (cc + 1), :])
                    qs[bi][cc] = qt
                if bi == 0:
                    onesb = sb.tile([1, 128], f16, tag="onesb")
                    nc.sync.dma_start(onesb, onesb_d[:])
                    onesr = sb.tile([128, 1], f16, tag="onesr")
                    nc.sync.dma_start(onesr, onesr_d[:])

            # PE warmup on a memset tile: long enough to run continuously
            # into the first Gram matmul so the p-state ramp completes before
            # real work starts (any idle gap resets the Tensor engine to a
            # 2x-slower state for its next 3us)
            junk = sb.tile([128, 128], f16, tag="junk")
            nc.vector.memset(junk, 0.0)
            warm = sps.tile([128, 128], f32, tag="wrm")
            for _ in range(N_WARM):
                nc.tensor.matmul(warm, junk, junk, start=True, stop=True,
                                 skip_group_check=True)

            # doubled v buffers: layout [128, NCC, 2L] so one dynamic-offset
            # AP can window all channel chunks at once
            vv = []
            for bi in range(BLOC):
                t = sb.tile([128, NCC, 2 * L], f16, tag=f"vv{bi}")
                for cc in range(NCC):
                    eng = nc.sync if cc % 2 == 0 else nc.scalar
                    eng.dma_start(t[:, cc, 0:L], vT[bi, 128 * cc:128 * (cc + 1), :])
                nc.vector.tensor_copy(t[:, :, L:2 * L], t[:, :, 0:L])
                vv.append(t)

            # ---- Gram with rotated PSUM accumulation; cc outermost so one
            # (k, q) chunk pair covers 12 s-blocks of matmul ----
            usegs = []
            for u in range(NJ):
                r = (L - 128 * u) % L
                segs = []
                t0 = 0
                while t0 < L:
                    y0 = (t0 + r) % L
                    seg = min(512 - (y0 % 512), L - t0, L - y0)
                    segs.append((t0, y0, seg))
                    t0 += seg
                usegs.append(segs)

            def gram_matmuls(bi, S, cc_range):
                for cc in cc_range:
                    for u in range(NJ):
                        for (ts_, ys_, seg) in usegs[u]:
                            nc.tensor.matmul(
                                S[:, ys_:ys_ + seg],
                                ks[bi][cc][:, 128 * u:128 * (u + 1)],
                                qs[bi][cc][:, ts_:ts_ + seg],
                                start=(u == 0 and cc == 0),
                                stop=(u == NJ - 1 and cc == NCC - 1),
                                skip_group_check=True,
                            )

            def evict_sdb(bi, S):
                # PSUM -> fp16 SBUF with the 1/HE scale fused, split across
                # ACT and DVE so both halves run concurrently; the wrapped
                # tail copy duplicates cols [0,128) at [L,L+128) so the
                # diagonal is one rectangle. The per-partition +1 element
                # skew is NOT expressible for compute engines or (in the
                # compiled DMA lowering) SBUF-side APs, so realign via DRAM:
                # write row p at flat offset (LD-1)*p (skewed -p per row),
                # read back with row stride LD -> diag[p, j] = sdb[p, p+j].
                # Rows overlap at one never-read address per pair (p<=127
                # keeps reads clear of it).
                sdb = sb.tile([128, LD], f16, tag=f"sdb{bi}")
                nc.scalar.mul(sdb[:, 0:L], S, 1.0 / C)
                nc.scalar.copy(sdb[:, L:LD], sdb[:, 0:128])
                # two independent DRAM staging tensors so the second write
                # chunk and first read-back chunk pipeline (a single tensor
                # would serialize on conservative whole-tile deps)
                skewA = dram.tile([128, LD], f16)
                skewB = dram.tile([128, LD], f16)
                ska, skb = skewA[:], skewB[:]
                nc.sync.dma_start(
                    bass.AP(ska.tensor, ska.offset, [(LD - 1, 128), (1, 896)]),
                    sdb[:, 0:896])
                nc.scalar.dma_start(
                    bass.AP(skb.tensor, skb.offset, [(LD - 1, 128), (1, LD - 768)]),
                    sdb[:, 768:LD])
                dg = sb.tile([128, L], f16, tag=f"diag{bi}")
                nc.sync.dma_start(
                    dg[:, 0:768],
                    bass.AP(ska.tensor, ska.offset, [(LD, 128), (1, 768)]))
                nc.scalar.dma_start(
                    dg[:, 768:L],
                    bass.AP(skb.tensor, skb.offset, [(LD, 128), (1, 768)]))

                def diag(ch):
                    return dg[:, 512 * ch:512 * (ch + 1)]
                return diag

            S0 = sps.tile([128, L], f32, tag="S0")
            gram_matmuls(0, S0, range(NCC))
            diag0 = evict_sdb(0, S0)

            S1 = sps.tile([128, L], f32, tag="S1")
            gram_matmuls(1, S1, range(0, 2))

            # mc0 = per-partition-reduced diag0, on the PE mid-Gram1
            # (S0's banks are free once its eviction ran)
            S0b = sps.tile([128, L], f32, tag="S0")
            for ch in range(NLT):
                nc.tensor.matmul(S0b[0:1, 512 * ch:512 * (ch + 1)], onesr,
                                 diag0(ch),
                                 start=True, stop=True, skip_group_check=True)
            mc0 = sb.tile([1, L], f32, tag="mc0")
            nc.scalar.copy(mc0, S0b[0:1, :])

            gram_matmuls(1, S1, range(2, NCC))
            diag1 = evict_sdb(1, S1)

            # gl = sum_p (diag0 + diag1): one PSUM accumulation group
            S1b = sps.tile([128, L], f32, tag="S1")
            for di, dg in enumerate((diag0, diag1)):
                for ch in range(NLT):
                    nc.tensor.matmul(S1b[0:1, 512 * ch:512 * (ch + 1)], onesr,
                                     dg(ch),
                                     start=(di == 0), stop=(di == 1),
                                     skip_group_check=True)

            S0c = sps.tile([128, L], f32, tag="S0")
            for ch in range(NLT):
                nc.tensor.matmul(S0c[0:1, 512 * ch:512 * (ch + 1)], onesr,
                                 diag1(ch),
                                 start=True, stop=True, skip_group_check=True)
            mc1 = sb.tile([1, L], f32, tag="mc1")
            nc.scalar.copy(mc1, S0c[0:1, :])

            # ---- global mean: cross-core AllReduce of gl ----
            if num_cores > 1:
                gl = sb.tile([1, L], f32, tag="gl")
                nc.scalar.copy(gl, S1b[0:1, :])
                cc_in = dram.tile([1, L], f32)
                cc_out = dram.tile([1, L], f32)
                nc.sync.dma_start(cc_in, gl)
                nc.gpsimd.collective_compute(
                    "AllReduce",
                    mybir.AluOpType.add,
                    replica_groups=[list(range(num_cores))],
                    ins=[cc_in.opt()],
                    outs=[cc_out.opt()],
                )
                gm = sb.tile([1, L], f32, tag="gm")
                nc.sync.dma_start(gm, cc_out)
            else:
                # single core: top-8 reads the PSUM accumulator directly
                gm = S1b[0:1, :]

            # ---- top-7 lags (top-8 instruction, first 7 used) ----
            vals = sb.tile([1, 8], f32, tag="vals")
            idxs = sb.tile([1, 8], u32, tag="idxs")
            nc.vector.max(vals, gm)
            nc.vector.max_index(idxs, vals, gm)
            if DEBUG_BUILD:
                nc.sync.dma_start(dbg_gm[:], gm)
                nc.sync.dma_start(dbg_idx[:], idxs)
                nc.sync.dma_start(dbg_mc[0:1, :], mc0)
                nc.sync.dma_start(dbg_mc[1:2, :], mc1)

            act_eng = nc.engines[ACT]
            dve_eng = nc.engines[DVE]
            pe_eng = nc.engines[PE]

            # PE tap offsets: x = (L - lag_k) + 512*lt
            sv_x = {}
            for k in PE_TAPS:
                rp = pe_eng.alloc_register(f"ip{k}")
                pe_eng.reg_load(rp, idxs[0:1, k:k + 1])
                ro = pe_eng.alloc_register(f"io{k}")
                pe_eng.reg_alu(ro, L, rp, mybir.AluOpType.subtract)
                for lt in range(NLT):
                    rx = pe_eng.alloc_register(f"ix{k}_{lt}")
                    pe_eng.reg_alu(rx, ro, 512 * lt, mybir.AluOpType.add)
                    sv_x[(k, lt)] = pe_eng.snap(rx, donate=True, min_val=1,
                                                max_val=L + 1024)
                pe_eng.free_register(ro)

            # DVE/ACT tap window offsets o = L - lag_k
            rp = dve_eng.alloc_register("dp5")
            dve_eng.reg_load(rp, idxs[0:1, B1_DVE_TAP:B1_DVE_TAP + 1])
            ro = dve_eng.alloc_register("do5")
            dve_eng.reg_alu(ro, L, rp, mybir.AluOpType.subtract)
            sv_dve_lt = []
            for lt in range(NLT):
                rx = dve_eng.alloc_register(f"dx{lt}")
                dve_eng.reg_alu(rx, ro, 512 * lt, mybir.AluOpType.add)
                sv_dve_lt.append(dve_eng.snap(rx, donate=True, min_val=1,
                                              max_val=L + 512 * lt))
            sv_dve = dve_eng.snap(ro, donate=True, min_val=1, max_val=L)
            rp = act_eng.alloc_register("ap6")
            act_eng.reg_load(rp, idxs[0:1, ACT_TAP:ACT_TAP + 1])
            ro = act_eng.alloc_register("ao6")
            act_eng.reg_alu(ro, L, rp, mybir.AluOpType.subtract)
            sv_act = act_eng.snap(ro, donate=True, min_val=1, max_val=L)

            # ---- weights: gather mean_corr[b, lag_k] straight from SBUF ----
            wr = []
            for bi in range(BLOC):
                wrt = sb.tile([1, 8], f32, tag=f"wr{bi}")
                wr.append(wrt)
            for k in range(TOPK):
                rg = dve_eng.alloc_register(f"dg{k}")
                dve_eng.reg_load(rg, idxs[0:1, k:k + 1])
                sv = dve_eng.snap(rg, donate=True, min_val=0, max_val=L - 1)
                nc.vector.tensor_copy(wr[0][0:1, k:k + 1],
                                      mc0[0:1, bass.ds(sv, 1)])
            for k in range(TOPK):
                rg = act_eng.alloc_register(f"ag{k}")
                act_eng.reg_load(rg, idxs[0:1, k:k + 1])
                sv = act_eng.snap(rg, donate=True, min_val=0, max_val=L - 1)
                nc.scalar.copy(wr[1][0:1, k:k + 1], mc1[0:1, bass.ds(sv, 1)])

            if DEBUG_BUILD:
                nc.sync.dma_start(dbg_wr[0:1, :], wr[0])
                nc.sync.dma_start(dbg_wr[1:2, :], wr[1])

            # softmax per batch (no max-subtraction: |mean_corr| is small and
            # fp32 exp is safe; softmax is shift-invariant)
            wbcs = []
            for bi in range(BLOC):
                ex = sb.tile([1, 8], f32, tag=f"ex{bi}")
                nc.scalar.activation(ex[0:1, 0:TOPK], wr[bi][0:1, 0:TOPK],
                                     mybir.ActivationFunctionType.Exp,
                                     bias=0.0, scale=1.0)
                sm = sb.tile([1, 1], f32, tag=f"sm{bi}")
                nc.vector.reduce_sum(sm, ex[0:1, 0:TOPK],
                                     axis=mybir.AxisListType.X)
                rs = sb.tile([1, 1], f32, tag=f"rs{bi}")
                nc.vector.reciprocal(rs, sm)
                w16 = sb.tile([1, 8], f16, tag=f"w16{bi}")
                nc.vector.tensor_scalar_mul(w16[0:1, 0:TOPK], ex[0:1, 0:TOPK],
                                            rs[0:1, 0:1])
                # broadcast to 128 partitions via ones-column matmul
                wps = sps.tile([128, 128], f32, tag="wps")
                nc.tensor.matmul(wps[:, 0:TOPK], onesb, w16[0:1, 0:TOPK],
                                 start=True, stop=True, skip_group_check=True)
                wbc = sb.tile([128, 8], f32, tag=f"wbc{bi}")
                nc.scalar.copy(wbc[:, 0:TOPK], wps[:, 0:TOPK])
                wbcs.append(wbc)

            # w-scaled identities for the PE taps
            Iw = [[None] * TOPK for _ in range(BLOC)]
            for bi in range(BLOC):
                taps = PE_TAPS if bi == 0 else PE_TAPS[:-1]
                for k in taps:
                    t = sb.tile([128, 128], f16, tag=f"iw{bi}{k}")
                    nc.vector.tensor_scalar_mul(t, ident, wbcs[bi][:, k:k + 1])
                    Iw[bi][k] = t

            # ---- weighted circular gather-sum ----
            # acc chains, chunked per channel chunk so cc0's eviction never
            # waits for later chunks: b0 = ACT stage only; b1 = ACT stage +
            # fused DVE tap (DVE chunks interleave behind b0's evictions)
            acc0 = sb.tile([128, NCC, L], f16, tag="acc0")
            for cc in range(NCC):
                nc.scalar.mul(acc0[:, cc, :],
                              vv[0][:, cc, bass.ds(sv_act, L)],
                              wbcs[0][:, ACT_TAP:ACT_TAP + 1])
            acc1s = sb.tile([128, NCC, L], f16, tag="acc1s")
            for cc in range(NCC):
                nc.scalar.mul(acc1s[:, cc, :],
                              vv[1][:, cc, bass.ds(sv_act, L)],
                              wbcs[1][:, ACT_TAP:ACT_TAP + 1])
            acc1 = sb.tile([128, NCC, L], f16, tag="acc1")
            accs = [acc0, acc1]

            # PE taps into PSUM; fused DVE eviction adds the chain acc;
            # b1's fused DVE tap chunks ride behind b0's evictions
            for bi in range(BLOC):
                taps = PE_TAPS if bi == 0 else PE_TAPS[:-1]
                cc_order = range(NCC) if bi == 0 else [0, 1, 3, 2]
                for cc in cc_order:
                    tgt = sps.tile([128, L], f32,
                                   tag=("S0" if cc % 2 == 0 else "S1"))
                    ot = obp.tile([128, L], f16, tag="ot")
                    fine = (bi == 1 and cc == NCC - 1) or (bi == 0 and cc < 1)
                    for lt in range(NLT):
                        for ki, k in enumerate(taps):
                            nc.tensor.matmul(
                                tgt[:, 512 * lt:512 * (lt + 1)],
                                Iw[bi][k],
                                vv[bi][:, cc, bass.ds(sv_x[(k, lt)], 512)],
                                start=(ki == 0),
                                stop=(ki == len(taps) - 1),
                                skip_group_check=True,
                            )
                        if fine:
                            nc.vector.scalar_tensor_tensor(
                                ot[:, 512 * lt:512 * (lt + 1)],
                                tgt[:, 512 * lt:512 * (lt + 1)],
                                1.0,
                                accs[bi][:, cc, 512 * lt:512 * (lt + 1)],
                                op0=MUL, op1=ADD)
                            eng = nc.sync if lt % 2 == 0 else nc.scalar
                            eng.dma_start(
                                out[bi, 128 * cc:128 * (cc + 1),
                                    512 * lt:512 * (lt + 1)],
                                ot[:, 512 * lt:512 * (lt + 1)],
                            )
                    if bi == 0:
                        nc.vector.scalar_tensor_tensor(
                            acc1[:, cc, :],
                            vv[1][:, cc, bass.ds(sv_dve, L)],
                            wbcs[1][:, B1_DVE_TAP:B1_DVE_TAP + 1],
                            acc1s[:, cc, :],
                            op0=MUL, op1=ADD)
                    if not fine:
                        # one coarse eviction per channel chunk: fewer DVE
                        # ops (DVE paces the output phase); stores stay
                        # sliced for DMA streaming
                        nc.vector.scalar_tensor_tensor(
                            ot, tgt, 1.0, accs[bi][:, cc, :],
                            op0=MUL, op1=ADD)
                        for lt in range(NLT):
                            eng = nc.sync if (cc * NLT + lt) % 2 == 0 else nc.scalar
                            eng.dma_start(
                                out[bi, 128 * cc:128 * (cc + 1),
                                    512 * lt:512 * (lt + 1)],
                                ot[:, 512 * lt:512 * (lt + 1)],
                            )
    nc.finalize()
    return nc


def _marshal(arr, ncores):
    # [B, L, H, E] fp32 -> per-core contiguous fp16 [BLOC, C, L]
    a = arr.reshape(B, L, C).astype(np.float16)
    a = np.ascontiguousarray(a.transpose(0, 2, 1))  # [B, C, L]
    bloc = B // ncores
    return [a[c * bloc:(c + 1) * bloc] for c in range(ncores)]


def _ensure_axon_hooks_importable():
    # some containers lack antenv.axon_hooks; run_bass_kernel_spmd imports it
    # unconditionally when tracing is requested. A None hook degrades to an
    # untraced run instead of crashing.
    import sys
    import types
    try:
        import antenv.axon_hooks  # noqa: F401
    except ModuleNotFoundError:
        try:
            import antenv
        except ModuleNotFoundError:
            return
        m = types.ModuleType("antenv.axon_hooks")
        m.get_axon_ntff_profile_hook = lambda: None
        sys.modules["antenv.axon_hooks"] = m
        antenv.axon_hooks = m


def kernel(queries, keys, values, attn_mask=None, _trace=False):
    from concourse.bass_utils import run_bass_kernel_spmd

    _ensure_axon_hooks_importable()

    nc = _cache.get("nc")
    if nc is None:
        nc = _build(NCORES)
        _cache["nc"] = nc

    qs = _marshal(np.asarray(queries, np.float32), NCORES)
    ks = _marshal(np.asarray(keys, np.float32), NCORES)
    vs = _marshal(np.asarray(values, np.float32), NCORES)
    in_maps = [{"qT": qs[c], "kT": ks[c], "vT": vs[c]} for c in range(NCORES)]

    res = run_bass_kernel_spmd(nc, in_maps, core_ids=list(range(NCORES)), trace=_trace)
    _cache["last"] = res
    o = np.concatenate([res.results[c]["out"] for c in range(NCORES)], axis=0)
    o = o.transpose(0, 2, 1).astype(np.float32)  # [B, L, C]
    return np.ascontiguousarray(o.reshape(B, L, H, E))
